# revision 54
# baseline (speedup 1.0000x reference)
"""Hetero GNN encoder/decoder (SAGE x2 + BN + edge MLP decoder) on 8 trn2 cores.

Strategy (edge sharding by destination, node-range sharding):
  - Articles: core k owns rows [k*APC, (k+1)*APC); customers likewise (CPC).
  - Message edges partitioned by dst-owner core; aggregation is computed
    fully locally via dma_gather (int16 per-src-block indices, max 1024
    idxs/instruction - ucode ring limit) + one-hot matmul (X^T @ P)
    scatter into PSUM windows of 128 nodes.  One-hot P is built with ONE
    is_equal op per gather chunk into a window-interleaved wide tile
    [P, TPC, MAXW*128] against a wide-iota constant; scatter matmuls are
    consolidated per (tile, PSUM-bank segment) with up-to-512-col spans
    (bank-level start/stop, per-element has_written accumulate).  The
    1/cnt mean scale is applied per dst NODE at the psum->meanT copy
    (tensor_tensor mult with host-replicated scale rows), not per edge.
  - After each SAGE layer, node features are AllGathered (row-major fp16
    tables) so the next layer / decoder can gather from the full table.
  - BatchNorm: local per-channel partial sums + tiny AllReduce.
  - Decoder uses precomputed U_c = bn(z_c) @ Wd1[:128] + b_dec1 and
    U_a = bn(z_a) @ Wd1[128:] tables; per label y = w2 . relu(U_c[lc]+U_a[la])
    + b2 - no PE work in the decoder loop at all.

All structure (loop bounds, window emissions) is compile-time and identical
across cores; per-core variation lives in the data (padded to uniform sizes).
"""
import os
import sys

sys.path.insert(0, "/opt/trn_rl_repo")

import numpy as np

import concourse.bacc as bacc
import concourse.bass as bass
import concourse.mybir as mybir
import concourse.tile as tile
from concourse.bass_utils import run_bass_kernel_spmd
from concourse.masks import make_identity

P = 128
NCORES = 8
GCH = int(os.environ.get("KGCH", "1024"))    # indices per agg dma_gather
TPC = GCH // P      # tiles per gather chunk
DGCH = int(os.environ.get("KDGCH", "1024"))  # indices per decoder dma_gather
DTPC = DGCH // P
SCRATCH = int(os.environ.get("KSCRATCH", "16384"))  # SWDGE desc ring bytes
WCH = 512           # W-stage column chunk
MAXW = 4            # max windows per tile (window-relative encoding)
BN_EPS = 1e-5


class Cfg:
    def __init__(self, n_c=300000, n_a=100000, e_lbl=1000000,
                 sbn=1280, srcb_c=30000, srcb_a=25000, f16=True):
        self.n_c, self.n_a, self.e_lbl = n_c, n_a, e_lbl
        self.cpc, self.apc = n_c // NCORES, n_a // NCORES
        assert self.cpc * NCORES == n_c and self.apc * NCORES == n_a
        self.chalf = self.cpc // 2
        assert self.chalf * 2 == self.cpc
        self.sbn = sbn
        self.srcb_c, self.srcb_a = srcb_c, srcb_a
        self.nblk_c = -(-n_c // srcb_c)
        self.nblk_a = -(-n_a // srcb_a)
        assert srcb_c < 32768 and srcb_a < 32768
        self.zc_sub = -(-self.cpc // 2)          # U_c local gather sub-block
        assert self.zc_sub < 32768
        self.f16 = f16
        self.dt = mybir.dt.float16 if f16 else mybir.dt.float32
        self.npdt = np.float16 if f16 else np.float32


def _ru(x, m):
    return (x + m - 1) // m * m


def _wrap_idx_chunk(flat):
    """[n] int16 -> [128, n/16] wrap (16-partition, replicated x8)."""
    n = flat.shape[0]
    w = flat.astype(np.int16).reshape(n // 16, 16).T
    return np.tile(w, (8, 1))


def _pack_pcol(a):
    """[n] -> [128, n/128]: element i -> partition i%128, col i//128."""
    return np.ascontiguousarray(a.reshape(-1, P).T)


# ---------------------------------------------------------------------------
# host-side structure + array prep for one aggregation pass
# ---------------------------------------------------------------------------

class AggPass:
    """Static structure (shared across cores) + per-core packed arrays."""

    def __init__(self, name, nloc, srcb, nsrc_blk, nsrc_rows, sbn):
        self.name = name
        self.nloc = nloc
        self.srcb = srcb
        self.nsrc_blk = nsrc_blk
        self.nsrc_rows = nsrc_rows
        self.sbn = sbn
        self.nsb = -(-nloc // sbn)
        self.sb_nodes = [min(sbn, nloc - s * sbn) for s in range(self.nsb)]
        self.run_L = None        # [nsb, nsrc_blk] uniform padded run lengths
        self.etot = 0
        self.emits = None        # per sb: list of (j, t, wlo, nwin) per tile
        self.uncovered = None    # per sb: list of never-touched windows
        self.idx = None          # per core [128, etot/16] int16
        self.dsc = None          # per core [128, etot/128] f16 (dst rel/run)


def prep_agg_pass(name, src, dst_loc, core_e, nloc, srcb, nsrc_blk,
                  nsrc_rows, sbn):
    ap = AggPass(name, nloc, srcb, nsrc_blk, nsrc_rows, sbn)
    nsb = ap.nsb
    nruns = nsb * nsrc_blk

    per_core = []
    counts = np.zeros((NCORES, nruns), np.int64)
    for k in range(NCORES):
        m = core_e == k
        s, d = src[m], dst_loc[m]
        j = s // srcb
        sb = d // sbn
        order = np.lexsort((d, j, sb))
        s, d, j, sb = s[order], d[order], j[order], sb[order]
        rid = sb * nsrc_blk + j
        counts[k] = np.bincount(rid, minlength=nruns)
        per_core.append((s, d, rid))

    run_L = _ru(counts.max(axis=0), P)          # uniform, 128-multiple
    offs = np.concatenate([[0], np.cumsum(run_L)]).astype(np.int64)
    etot = int(offs[-1])
    ap.run_L = run_L.reshape(nsb, nsrc_blk)
    ap.etot = etot

    # superblock-relative dst per core (for window structure)
    dstrel_all = np.full((NCORES, etot), -1.0e9, np.float64)
    pos_all = []
    for k in range(NCORES):
        s, d, rid = per_core[k]
        run_start = np.concatenate([[0], np.cumsum(counts[k])])[:-1]
        pos = offs[rid] + (np.arange(len(s)) - run_start[rid])
        pos_all.append(pos)
        dstrel_all[k, pos] = (d - (d // sbn) * sbn).astype(np.float64)

    # per-tile window range (union over cores)
    T = etot // P
    Dw = dstrel_all.reshape(NCORES, T, P)
    valid_any = Dw.max(axis=2) >= 0              # [NCORES, T]
    wlo_c = np.where(Dw >= 0, Dw, np.inf).min(axis=2) // P
    whi_c = np.where(Dw >= 0, Dw, -np.inf).max(axis=2) // P
    wlo_t = np.where(valid_any, wlo_c, np.inf).min(axis=0)
    whi_t = np.where(valid_any, whi_c, -np.inf).max(axis=0)

    emits = []
    uncovered = []
    tile_wlo = np.zeros(T, np.int64)             # per global tile
    for s in range(nsb):
        nwin = -(-ap.sb_nodes[s] // P)
        covered = set()
        sb_emits = []
        for j in range(nsrc_blk):
            r = s * nsrc_blk + j
            o = int(offs[r])
            nt = int(ap.run_L[s, j] // P)
            for t in range(nt):
                g = o // P + t
                if np.isfinite(wlo_t[g]):
                    a = max(0, min(int(wlo_t[g]), nwin - 1))
                    b = max(a, min(int(whi_t[g]), nwin - 1))
                else:
                    a, b = 0, 0
                nw = b - a + 1
                assert nw <= MAXW, f"tile spans {nw} windows"
                covered.update(range(a, b + 1))
                tile_wlo[g] = a
                sb_emits.append((j, t, a, nw))
        emits.append(sb_emits)
        uncovered.append(sorted(set(range(nwin)) - covered))
    ap.emits = emits
    ap.uncovered = uncovered

    # per-core packed arrays (dst window-relative to tile_wlo)
    idxs, dscs = [], []
    for k in range(NCORES):
        s, d, rid = per_core[k]
        pos = pos_all[k]
        idx16 = np.zeros(etot, np.int16)
        idx16[pos] = (s - (s // srcb) * srcb).astype(np.int16)
        dstrel = np.full(etot, -1000.0, np.float32)
        dstrel[pos] = (d - (d // sbn) * sbn).astype(np.float32)
        dstrel -= 128.0 * tile_wlo[np.arange(etot) // P]
        dstrel[dstrel < -1000.0] = -1000.0

        wrapped = np.zeros((P, etot // 16), np.int16)
        dsc = np.zeros((P, etot // P), np.float16)
        for r in range(nruns):
            o = int(offs[r])
            L = int(run_L[r])
            if L == 0:
                continue
            wrapped[:, o // 16:(o + L) // 16] = _wrap_idx_chunk(idx16[o:o + L])
            nt = L // P
            c0 = o // P
            dsc[:, c0:c0 + nt] = _pack_pcol(dstrel[o:o + L]).astype(np.float16)
        idxs.append(wrapped)
        dscs.append(dsc)
    ap.idx, ap.dsc = idxs, dscs
    return ap


# ---------------------------------------------------------------------------
# full host prep
# ---------------------------------------------------------------------------

def prep_all(cfg, inputs):
    i64 = lambda a: np.asarray(a).astype(np.int64)
    e_src = i64(inputs["edge_src_customer"])
    e_dst = i64(inputs["edge_dst_article"])
    l_c = i64(inputs["label_customer"])
    l_a = i64(inputs["label_article"])

    cnt_a = np.bincount(e_dst, minlength=cfg.n_a)
    cnt_c = np.bincount(e_src, minlength=cfg.n_c)
    scl_a = (1.0 / np.maximum(cnt_a, 1.0)).astype(np.float16)
    scl_c = (1.0 / np.maximum(cnt_c, 1.0)).astype(np.float16)

    pa = prep_agg_pass(
        "A", e_src, e_dst % cfg.apc, e_dst // cfg.apc,
        cfg.apc, cfg.srcb_c, cfg.nblk_c, cfg.n_c, cfg.sbn)
    dloc = e_src % cfg.cpc
    core_c = e_src // cfg.cpc
    half = (dloc >= cfg.chalf).astype(np.int64)
    pcs = []
    for h in range(2):
        m = half == h
        pcs.append(prep_agg_pass(
            f"C{h}", e_dst[m], dloc[m] - h * cfg.chalf,
            core_c[m], cfg.chalf, cfg.srcb_a, cfg.nblk_a, cfg.n_a, cfg.sbn))
    scls = dict(scl_a=scl_a, scl_c=scl_c)

    # decoder labels: partition by customer core, group by (sub, ablk)
    core_l = l_c // cfg.cpc
    sub_l = (l_c % cfg.cpc) // cfg.zc_sub
    ablk_l = l_a // cfg.srcb_a
    gid = sub_l * cfg.nblk_a + ablk_l
    ngrp = 2 * cfg.nblk_a
    gcounts = np.zeros((NCORES, ngrp), np.int64)
    per_core_lbl = []
    for k in range(NCORES):
        m = core_l == k
        lc, la, g, orig = l_c[m], l_a[m], gid[m], np.nonzero(m)[0]
        order = np.argsort(g, kind="stable")
        lc, la, g, orig = lc[order], la[order], g[order], orig[order]
        gcounts[k] = np.bincount(g, minlength=ngrp)
        per_core_lbl.append((lc, la, g, orig))
    grp_L = _ru(gcounts.max(axis=0), P)
    goffs = np.concatenate([[0], np.cumsum(grp_L)]).astype(np.int64)
    ld_pad = int(goffs[-1])

    dec_idx_c, dec_idx_a, out_pos = [], [], []
    for k in range(NCORES):
        lc, la, g, orig = per_core_lbl[k]
        gstart = np.concatenate([[0], np.cumsum(gcounts[k])])[:-1]
        pos = goffs[g] + (np.arange(len(lc)) - gstart[g])
        ic = np.zeros(ld_pad, np.int16)
        ia = np.zeros(ld_pad, np.int16)
        po = np.full(ld_pad, -1, np.int64)
        lcl = lc % cfg.cpc
        ic[pos] = (lcl - (lcl // cfg.zc_sub) * cfg.zc_sub).astype(np.int16)
        ia[pos] = (la - (la // cfg.srcb_a) * cfg.srcb_a).astype(np.int16)
        po[pos] = orig
        wc = np.zeros((P, ld_pad // 16), np.int16)
        wa = np.zeros((P, ld_pad // 16), np.int16)
        for gi in range(ngrp):
            o = int(goffs[gi])
            L = int(grp_L[gi])
            if L:
                wc[:, o // 16:(o + L) // 16] = _wrap_idx_chunk(ic[o:o + L])
                wa[:, o // 16:(o + L) // 16] = _wrap_idx_chunk(ia[o:o + L])
        dec_idx_c.append(wc)
        dec_idx_a.append(wa)
        out_pos.append(po)

    dec = dict(grp_L=grp_L.reshape(2, cfg.nblk_a), goffs=goffs, ld_pad=ld_pad,
               idx_c=dec_idx_c, idx_a=dec_idx_a, out_pos=out_pos)
    return pa, pcs, dec, scls


# ---------------------------------------------------------------------------
# kernel builder
# ---------------------------------------------------------------------------

F32 = mybir.dt.float32


def build_nc(cfg, pa, pcs, dec, dbg=False):
    DT = cfg.dt
    nc = bacc.Bacc("TRN2", target_bir_lowering=False, debug=False,
                   num_devices=NCORES, num_swdge_queues=4,
                   dynamic_dma_scratch_size=SCRATCH)
    qctr = [0]
    def next_q():
        qctr[0] = (qctr[0] + 1) % 4
        return qctr[0]

    ei = lambda n, s, d: nc.dram_tensor(n, s, d, kind="ExternalInput")
    xc = ei("xc", [cfg.n_c, P], DT)
    xa = ei("xa", [cfg.n_a, P], DT)
    xaT = ei("xaT", [P, cfg.apc], DT)
    xcT = ei("xcT", [P, cfg.cpc], DT)
    aggA_idx = ei("aggA_idx", [P, pa.etot // 16], mybir.dt.int16)
    aggA_dsc = ei("aggA_dsc", [P, pa.etot // P], DT)
    aggC_idx = [ei(f"aggC{h}_idx", [P, pcs[h].etot // 16], mybir.dt.int16)
                for h in range(2)]
    aggC_dsc = [ei(f"aggC{h}_dsc", [P, pcs[h].etot // P], DT)
                for h in range(2)]
    sclA = ei("sclA", [P, cfg.apc], DT)
    sclC = ei("sclC", [P, cfg.cpc], DT)
    dec_idx_c = ei("dec_idx_c", [P, dec["ld_pad"] // 16], mybir.dt.int16)
    dec_idx_a = ei("dec_idx_a", [P, dec["ld_pad"] // 16], mybir.dt.int16)

    wnames = ["W_msg1_ca", "W_self1_a", "W_msg1_ac", "W_self1_c",
              "W_msg2_ca", "W_self2_a", "W_msg2_ac", "W_self2_c",
              "Wd1c", "Wd1a"]
    wts = {n: ei(n, [P, P], DT) for n in wnames}
    w2rep = ei("w2rep", [P, DGCH], DT)      # W_dec2 repeated per 128-segment
    bnames = ["b1_a", "b1_c", "b2_a", "b2_c",
              "bn_gamma_c", "bn_beta_c", "bn_gamma_a", "bn_beta_a",
              "b_dec1", "b_dec2c"]
    bis = {n: ei(n, [P, 1], F32) for n in bnames}

    ldT = dec["ld_pad"] // P
    y_out = nc.dram_tensor("y", [P, ldT], F32, kind="ExternalOutput")
    dbg_outs = {}
    if dbg:
        dbg_outs = {
            "d_ha": nc.dram_tensor("d_ha", [cfg.n_a, P], DT,
                                   kind="ExternalOutput"),
            "d_hc": nc.dram_tensor("d_hc", [cfg.n_c, P], DT,
                                   kind="ExternalOutput"),
            "d_ua": nc.dram_tensor("d_ua", [cfg.n_a, P], DT,
                                   kind="ExternalOutput"),
            "d_uc": nc.dram_tensor("d_uc", [cfg.cpc, P], DT,
                                   kind="ExternalOutput"),
            "d_st": nc.dram_tensor("d_st", [P, 4], F32,
                                   kind="ExternalOutput"),
        }

    rg = [list(range(NCORES))]

    with tile.TileContext(nc) as tc:
        with (
            tc.tile_pool(name="dramp", bufs=1, space="DRAM") as dramp,
            tc.tile_pool(name="const", bufs=1) as cs,
        ):
            ha_own = dramp.tile([cfg.apc, P], DT)
            ha_full = dramp.tile([cfg.n_a, P], DT, addr_space="Shared")
            hc_own = dramp.tile([cfg.cpc, P], DT)
            hc_full = dramp.tile([cfg.n_c, P], DT, addr_space="Shared")
            ua_own = dramp.tile([cfg.apc, P], DT)
            ua_full = dramp.tile([cfg.n_a, P], DT, addr_space="Shared")
            uc_loc = dramp.tile([cfg.cpc, P], DT)
            haT_d = dramp.tile([P, cfg.apc], DT)
            hcT_d = dramp.tile([P, cfg.cpc], DT)
            zaT_d = dramp.tile([P, cfg.apc], DT)
            zcT_d = dramp.tile([P, cfg.cpc], DT)
            stats_in_a = dramp.tile([P, 2], F32)
            stats_out_a = dramp.tile([P, 2], F32, addr_space="Shared")
            stats_in_c = dramp.tile([P, 2], F32)
            stats_out_c = dramp.tile([P, 2], F32, addr_space="Shared")

            # constant: per-tile window-span iota [P, TPC, MAXW*P], value=w
            iota_big = cs.tile([P, TPC, MAXW * P], DT, name="iobig")
            with tc.tile_pool(name="iotmp", bufs=1) as iop:
                ii = iop.tile([P, TPC, MAXW * P], mybir.dt.int32, name="ioi")
                nc.gpsimd.iota(ii[:], pattern=[[0, TPC], [1, MAXW * P]],
                               base=0, channel_multiplier=0)
                nc.vector.tensor_copy(iota_big[:], ii[:])
            ident = cs.tile([P, P], DT)
            make_identity(nc, ident[:])
            w_sb = {n: cs.tile([P, P], DT, name=f"w_{n}") for n in wnames}
            for n in wnames:
                nc.sync.dma_start(out=w_sb[n][:], in_=wts[n][:])
            w2r_sb = cs.tile([P, DGCH], DT)
            nc.sync.dma_start(out=w2r_sb[:], in_=w2rep[:])
            b_sb = {n: cs.tile([P, 1], F32, name=f"b_{n}") for n in bnames}
            for n in bnames:
                nc.sync.dma_start(out=b_sb[n][:], in_=bis[n][:])
            stats_sb = cs.tile([P, 4], F32)
            nc.vector.memset(stats_sb[:], 0.0)

            # ---------------- aggregation pass ----------------
            def agg_pass(ps, table, idx_d, dsc_d, meanT_sb, scl_d, scl_off):
                offs = np.concatenate(
                    [[0], np.cumsum(ps.run_L.reshape(-1))]).astype(np.int64)
                with (
                    tc.tile_pool(name=f"ag_{ps.name}", bufs=1) as sbp,
                    tc.tile_pool(name=f"agp_{ps.name}", bufs=2,
                                 space="PSUM") as psp,
                ):
                    for s in range(ps.nsb):
                        nodes = ps.sb_nodes[s]
                        nwin = -(-nodes // P)
                        psum = psp.tile([P, nwin * P], F32, tag="aggps",
                                        name="psum_agg", bufs=2)
                        scl_sb = sbp.tile([P, ps.sbn], DT, tag="gscl",
                                          name="gscl", bufs=2)
                        nc.sync.dma_start(
                            out=scl_sb[:, :nodes],
                            in_=scl_d[:, scl_off + s * ps.sbn:
                                      scl_off + s * ps.sbn + nodes])
                        # first/last toucher (j, t) per PSUM bank (4 win/bank)
                        touch = {}
                        for (j, t, wlo, nw) in ps.emits[s]:
                            for b in range(wlo // 4, (wlo + nw - 1) // 4 + 1):
                                touch.setdefault(b, []).append((j, t))
                        firsts = {b: v[0] for b, v in touch.items()}
                        lasts = {b: v[-1] for b, v in touch.items()}

                        # group emits per run
                        by_run = {}
                        for e in ps.emits[s]:
                            by_run.setdefault(e[0], []).append(e)
                        for j in sorted(by_run):
                            r = s * ps.nsrc_blk + j
                            o = int(offs[r])
                            L = int(ps.run_L[s, j])
                            nt = L // P
                            blk_rows = min(ps.srcb,
                                           ps.nsrc_rows - j * ps.srcb)
                            idx_sb = sbp.tile([P, L // 16], mybir.dt.int16,
                                              tag="gidx", name="gidx",
                                              bufs=6)
                            nc.sync.dma_start(
                                out=idx_sb[:],
                                in_=idx_d[:, o // 16:(o + L) // 16])
                            dsc_sb = sbp.tile([P, nt], DT, tag="gdsc",
                                              name="gdsc", bufs=6)
                            nc.sync.dma_start(
                                out=dsc_sb[:],
                                in_=dsc_d[:, o // P:o // P + nt])
                            x_tiles = []
                            for c0 in range(0, L, GCH):
                                cl = min(GCH, L - c0)
                                x = sbp.tile([P, TPC, P], DT, tag="gx",
                                             name="gx", bufs=10)
                                nc.gpsimd.dma_gather(
                                    x[:, :cl // P, :],
                                    table[j * ps.srcb:
                                          j * ps.srcb + blk_rows, :],
                                    idx_sb[:, c0 // 16:(c0 + cl) // 16],
                                    cl, cl, P, queue_num=next_q())
                                x_tiles.append(x)
                            # per gather-chunk batched one-hot builds into a
                            # window-interleaved wide tile [P, TPC, MAXW*P]
                            run_emits = by_run[j]
                            p8s = {}     # chunk -> wide tile
                            for c in range(0, nt, TPC):
                                ctn = min(TPC, nt - c)
                                maxnw = max(e[3] for e in run_emits
                                            if c <= e[1] < c + ctn)
                                dstb = dsc_sb[:, c:c + ctn] \
                                    .to_broadcast([P, ctn, MAXW * P])
                                p8 = sbp.tile([P, TPC, MAXW * P], DT,
                                              tag="gp", name="gp", bufs=4)
                                # full-width build: contiguous out/in0 APs
                                # run ~2x faster than partial strided writes
                                nc.vector.tensor_tensor(
                                    out=p8[:, :ctn, :],
                                    in0=iota_big[:, :ctn, :],
                                    in1=dstb,
                                    op=mybir.AluOpType.is_equal)
                                p8s[c // TPC] = p8
                            for (j2, t, wlo, nw) in run_emits:
                                p8 = p8s[t // TPC]
                                # split window span at PSUM bank boundaries
                                wa = wlo
                                while wa < wlo + nw:
                                    wb = min(wlo + nw - 1,
                                             (wa // 4) * 4 + 3)
                                    b = wa // 4
                                    nc.tensor.matmul(
                                        psum[:, wa * P:(wb + 1) * P],
                                        lhsT=x_tiles[t // TPC]
                                            [:, t % TPC, :],
                                        rhs=p8[:, t % TPC,
                                               (wa - wlo) * P:
                                               (wb - wlo + 1) * P],
                                        start=(firsts[b] == (j2, t)),
                                        stop=(lasts[b] == (j2, t)),
                                        skip_group_check=True)
                                    wa = wb + 1
                        nc.vector.tensor_tensor(
                            out=meanT_sb[:, s * ps.sbn:s * ps.sbn + nodes],
                            in0=psum[:, :nodes],
                            in1=scl_sb[:, :nodes],
                            op=mybir.AluOpType.mult)
                        for w in ps.uncovered[s]:
                            a = s * ps.sbn + w * P
                            b = min(a + P, s * ps.sbn + nodes)
                            nc.vector.memset(meanT_sb[:, a:b], 0.0)

            # ---------------- W stage ----------------
            def w_stage(nloc, meanT_sb, selfT_dram, self_off, wmsg, wself,
                        bias_col, relu, outT_dram, outT_off, rows_dram,
                        rows_off, stats_cols, sbp, psp):
                for c0 in range(0, nloc, WCH):
                    cw = min(WCH, nloc - c0)
                    sT = sbp.tile([P, WCH], DT, tag="wself", name="wselfT",
                                  bufs=3)
                    nc.sync.dma_start(
                        out=sT[:, :cw],
                        in_=selfT_dram[:, self_off + c0:self_off + c0 + cw])
                    psum = psp.tile([P, WCH], F32, tag="wps", name="wps",
                                    bufs=3)
                    nc.tensor.matmul(psum[:, :cw], lhsT=wmsg,
                                     rhs=meanT_sb[:, c0:c0 + cw],
                                     start=True, stop=False,
                                     skip_group_check=True)
                    nc.tensor.matmul(psum[:, :cw], lhsT=wself,
                                     rhs=sT[:, :cw],
                                     start=False, stop=True,
                                     skip_group_check=True)
                    oT = sbp.tile([P, WCH], DT, tag="woT", name="woT", bufs=3)
                    nc.scalar.activation(
                        oT[:, :cw], psum[:, :cw],
                        mybir.ActivationFunctionType.Relu if relu
                        else mybir.ActivationFunctionType.Identity,
                        bias=bias_col[:], scale=1.0)
                    nc.sync.dma_start(
                        out=outT_dram[:, outT_off + c0:outT_off + c0 + cw],
                        in_=oT[:, :cw])
                    if stats_cols is not None:
                        si, sj = stats_cols
                        part = sbp.tile([P, 1], F32, tag="wst1", name="wst1",
                                        bufs=2)
                        nc.vector.reduce_sum(part[:], oT[:, :cw],
                                             mybir.AxisListType.X)
                        nc.vector.tensor_add(stats_sb[:, si:si + 1],
                                             stats_sb[:, si:si + 1], part[:])
                        trash = sbp.tile([P, WCH], F32, tag="wtrash",
                                         name="wtrash", bufs=2)
                        part2 = sbp.tile([P, 1], F32, tag="wst2", name="wst2",
                                         bufs=2)
                        nc.scalar.activation(
                            trash[:, :cw], oT[:, :cw],
                            mybir.ActivationFunctionType.Square,
                            accum_out=part2[:])
                        nc.vector.tensor_add(stats_sb[:, sj:sj + 1],
                                             stats_sb[:, sj:sj + 1],
                                             part2[:])
                    if rows_dram is not None:
                        _emit_rows(oT, cw, rows_dram, rows_off + c0, sbp, psp)

            def _emit_rows(srcT_sb, cw, rows_dram, row_base, sbp, psp):
                for b0 in range(0, cw, P):
                    bw = min(P, cw - b0)
                    tp = psp.tile([P, P], DT, tag="tps", name="tps", bufs=2)
                    nc.tensor.transpose(tp[:bw, :], srcT_sb[:, b0:b0 + bw],
                                        ident[:])
                    rows = sbp.tile([P, P], DT, tag="rows", name="rows",
                                    bufs=3)
                    nc.scalar.copy(rows[:bw, :], tp[:bw, :])
                    nc.sync.dma_start(
                        out=rows_dram[row_base + b0:row_base + b0 + bw, :],
                        in_=rows[:bw, :])

            # ================= layer 1 =================
            with tc.tile_pool(name="meanA", bufs=1) as mp:
                meanT = mp.tile([P, pa.nsb * pa.sbn], DT, name="meanTA")
                agg_pass(pa, xc, aggA_idx, aggA_dsc, meanT, sclA, 0)
                with (
                    tc.tile_pool(name="w1a", bufs=1) as sbp,
                    tc.tile_pool(name="w1ap", bufs=1, space="PSUM") as psp,
                ):
                    w_stage(cfg.apc, meanT, xaT, 0, w_sb["W_msg1_ca"][:],
                            w_sb["W_self1_a"][:], b_sb["b1_a"], True,
                            haT_d, 0, ha_own, 0, None, sbp, psp)
            nc.gpsimd.collective_compute(
                "AllGather", mybir.AluOpType.bypass, replica_groups=rg,
                ins=[ha_own[:]], outs=[ha_full[:]])

            for h in range(2):
                with tc.tile_pool(name=f"meanC{h}", bufs=1) as mp:
                    meanT = mp.tile([P, pcs[h].nsb * pcs[h].sbn], DT,
                                    name="meanTC")
                    agg_pass(pcs[h], xa, aggC_idx[h], aggC_dsc[h], meanT,
                             sclC, h * cfg.chalf)
                    with (
                        tc.tile_pool(name=f"w1c{h}", bufs=1) as sbp,
                        tc.tile_pool(name=f"w1cp{h}", bufs=1,
                                     space="PSUM") as psp,
                    ):
                        w_stage(cfg.chalf, meanT, xcT, h * cfg.chalf,
                                w_sb["W_msg1_ac"][:], w_sb["W_self1_c"][:],
                                b_sb["b1_c"], True, hcT_d, h * cfg.chalf,
                                hc_own, h * cfg.chalf, None, sbp, psp)

            nc.gpsimd.collective_compute(
                "AllGather", mybir.AluOpType.bypass, replica_groups=rg,
                ins=[hc_own[:]], outs=[hc_full[:]])

            # ================= layer 2 (C first: hides hc AllGather) ======
            for h in range(2):
                with tc.tile_pool(name=f"meanC2{h}", bufs=1) as mp:
                    meanT = mp.tile([P, pcs[h].nsb * pcs[h].sbn], DT,
                                    name="meanTC2")
                    agg_pass(pcs[h], ha_full, aggC_idx[h], aggC_dsc[h],
                             meanT, sclC, h * cfg.chalf)
                    with (
                        tc.tile_pool(name=f"w2c{h}", bufs=1) as sbp,
                        tc.tile_pool(name=f"w2cp{h}", bufs=1,
                                     space="PSUM") as psp,
                    ):
                        w_stage(cfg.chalf, meanT, hcT_d, h * cfg.chalf,
                                w_sb["W_msg2_ac"][:], w_sb["W_self2_c"][:],
                                b_sb["b2_c"], False, zcT_d, h * cfg.chalf,
                                None, 0, (2, 3), sbp, psp)
            nc.sync.dma_start(out=stats_in_c[:], in_=stats_sb[:, 2:4])
            nc.gpsimd.collective_compute(
                "AllReduce", mybir.AluOpType.add, replica_groups=rg,
                ins=[stats_in_c[:]], outs=[stats_out_c[:]])

            with tc.tile_pool(name="meanA2", bufs=1) as mp:
                meanT = mp.tile([P, pa.nsb * pa.sbn], DT, name="meanTA2")
                agg_pass(pa, hc_full, aggA_idx, aggA_dsc, meanT, sclA, 0)
                with (
                    tc.tile_pool(name="w2a", bufs=1) as sbp,
                    tc.tile_pool(name="w2ap", bufs=1, space="PSUM") as psp,
                ):
                    w_stage(cfg.apc, meanT, haT_d, 0, w_sb["W_msg2_ca"][:],
                            w_sb["W_self2_a"][:], b_sb["b2_a"], False,
                            zaT_d, 0, None, 0, (0, 1), sbp, psp)
            nc.sync.dma_start(out=stats_in_a[:], in_=stats_sb[:, 0:2])
            nc.gpsimd.collective_compute(
                "AllReduce", mybir.AluOpType.add, replica_groups=rg,
                ins=[stats_in_a[:]], outs=[stats_out_a[:]])

            # ================= BN + U tables =================
            with (
                tc.tile_pool(name="bn", bufs=1) as sbp,
                tc.tile_pool(name="bnp", bufs=1, space="PSUM") as psp,
            ):
                st_c = sbp.tile([P, 2], F32, name="st_c")
                nc.sync.dma_start(out=st_c[:], in_=stats_out_c[:])
                st_a = sbp.tile([P, 2], F32, name="st_a")
                nc.sync.dma_start(out=st_a[:], in_=stats_out_a[:])

                def bn_coeff(st, si, sj, n, gamma, beta, tagp):
                    mu = sbp.tile([P, 1], F32, name=f"mu{tagp}")
                    nc.vector.tensor_scalar_mul(mu[:], st[:, si:si + 1],
                                                1.0 / n)
                    msq = sbp.tile([P, 1], F32, name=f"msq{tagp}")
                    nc.vector.tensor_scalar_mul(msq[:], st[:, sj:sj + 1],
                                                1.0 / n)
                    mu2 = sbp.tile([P, 1], F32, name=f"mu2{tagp}")
                    nc.vector.tensor_mul(mu2[:], mu[:], mu[:])
                    var = sbp.tile([P, 1], F32, name=f"var{tagp}")
                    nc.vector.tensor_sub(var[:], msq[:], mu2[:])
                    nc.vector.tensor_scalar_add(var[:], var[:], BN_EPS)
                    sd = sbp.tile([P, 1], F32, name=f"sd{tagp}")
                    nc.scalar.activation(sd[:], var[:],
                                         mybir.ActivationFunctionType.Sqrt)
                    rstd = sbp.tile([P, 1], F32, name=f"rstd{tagp}")
                    nc.vector.reciprocal(rstd[:], sd[:])
                    scl = sbp.tile([P, 1], F32, name=f"scl{tagp}")
                    nc.vector.tensor_mul(scl[:], b_sb[gamma][:], rstd[:])
                    mg = sbp.tile([P, 1], F32, name=f"mg{tagp}")
                    nc.vector.tensor_mul(mg[:], mu[:], scl[:])
                    bia = sbp.tile([P, 1], F32, name=f"bia{tagp}")
                    nc.vector.tensor_sub(bia[:], b_sb[beta][:], mg[:])
                    return scl, bia

                scl_c_c, bia_c_c = bn_coeff(st_c, 0, 1, cfg.n_c,
                                            "bn_gamma_c", "bn_beta_c", "c")
                scl_a_c, bia_a_c = bn_coeff(st_a, 0, 1, cfg.n_a,
                                            "bn_gamma_a", "bn_beta_a", "a")

                def bn_u(nloc, zT_dram, scl, bia, w1half, ubias, rows_dram):
                    """rows_dram <- rows of bn(z) @ w1half (+ubias)."""
                    for c0 in range(0, nloc, WCH):
                        cw = min(WCH, nloc - c0)
                        zT = sbp.tile([P, WCH], DT, tag="bnz", name="bnz",
                                      bufs=3)
                        nc.sync.dma_start(out=zT[:, :cw],
                                          in_=zT_dram[:, c0:c0 + cw])
                        bnT = sbp.tile([P, WCH], DT, tag="bnt", name="bnt",
                                       bufs=3)
                        nc.scalar.activation(
                            bnT[:, :cw], zT[:, :cw],
                            mybir.ActivationFunctionType.Identity,
                            bias=bia[:], scale=scl[:])
                        ups = psp.tile([P, WCH], F32, tag="ups", name="ups",
                                       bufs=2)
                        nc.tensor.matmul(ups[:, :cw], lhsT=w1half,
                                         rhs=bnT[:, :cw], start=True,
                                         stop=True, skip_group_check=True)
                        uT = sbp.tile([P, WCH], DT, tag="uT", name="uT",
                                      bufs=3)
                        nc.scalar.activation(
                            uT[:, :cw], ups[:, :cw],
                            mybir.ActivationFunctionType.Identity,
                            bias=ubias[:] if ubias is not None else 0.0,
                            scale=1.0)
                        _emit_rows(uT, cw, rows_dram, c0, sbp, psp)

                bn_u(cfg.cpc, zcT_d, scl_c_c, bia_c_c, w_sb["Wd1c"][:],
                     b_sb["b_dec1"], uc_loc)
                bn_u(cfg.apc, zaT_d, scl_a_c, bia_a_c, w_sb["Wd1a"][:],
                     None, ua_own)
            nc.gpsimd.collective_compute(
                "AllGather", mybir.AluOpType.bypass, replica_groups=rg,
                ins=[ua_own[:]], outs=[ua_full[:]])

            # ================= decoder =================
            grp_L = dec["grp_L"]
            goffs = dec["goffs"]
            with tc.tile_pool(name="dec", bufs=1) as sbp:
                ysb = sbp.tile([P, ldT], F32, name="ysb")
                for sub in range(2):
                    for ab in range(cfg.nblk_a):
                        gi = sub * cfg.nblk_a + ab
                        L = int(grp_L[sub, ab])
                        o = int(goffs[gi])
                        uc_rows = min(cfg.zc_sub, cfg.cpc - sub * cfg.zc_sub)
                        ua_rows = min(cfg.srcb_a, cfg.n_a - ab * cfg.srcb_a)
                        for c0 in range(0, L, DGCH):
                            cl = min(DGCH, L - c0)
                            oc = o + c0
                            ctn = cl // P
                            ixc = sbp.tile([P, DGCH // 16], mybir.dt.int16,
                                           tag="dixc", name="dixc", bufs=4)
                            nc.sync.dma_start(
                                out=ixc[:, :cl // 16],
                                in_=dec_idx_c[:, oc // 16:(oc + cl) // 16])
                            ucg = sbp.tile([P, DTPC, P], DT, tag="duc",
                                           name="duc", bufs=4)
                            nc.gpsimd.dma_gather(
                                ucg[:, :ctn, :],
                                uc_loc[sub * cfg.zc_sub:
                                       sub * cfg.zc_sub + uc_rows, :],
                                ixc[:, :cl // 16], cl, cl, P,
                                queue_num=next_q())
                            ixa = sbp.tile([P, DGCH // 16], mybir.dt.int16,
                                           tag="dixa", name="dixa", bufs=4)
                            nc.sync.dma_start(
                                out=ixa[:, :cl // 16],
                                in_=dec_idx_a[:, oc // 16:(oc + cl) // 16])
                            uag = sbp.tile([P, DTPC, P], DT, tag="dua",
                                           name="dua", bufs=4)
                            nc.gpsimd.dma_gather(
                                uag[:, :ctn, :],
                                ua_full[ab * cfg.srcb_a:
                                        ab * cfg.srcb_a + ua_rows, :],
                                ixa[:, :cl // 16], cl, cl, P,
                                queue_num=next_q())
                            ssum = sbp.tile([P, DGCH], DT, tag="dsum",
                                            name="dsum", bufs=4)
                            ssum3 = ssum[:, :cl].rearrange(
                                "p (t w) -> p t w", w=P)
                            nc.vector.tensor_tensor(
                                out=ssum3,
                                in0=ucg[:, :ctn, :], in1=uag[:, :ctn, :],
                                op=mybir.AluOpType.add)
                            nc.vector.scalar_tensor_tensor(
                                out=ssum[:, :cl], in0=ssum[:, :cl],
                                scalar=0.0, in1=w2r_sb[:, :cl],
                                op0=mybir.AluOpType.max,
                                op1=mybir.AluOpType.mult)
                            nc.vector.reduce_sum(
                                ysb[:, oc // P:oc // P + ctn],
                                ssum[:, :cl].rearrange(
                                    "p (t w) -> p t w", w=P),
                                mybir.AxisListType.X)
                nc.vector.tensor_scalar(
                    out=ysb[:], in0=ysb[:], scalar1=b_sb["b_dec2c"][:],
                    scalar2=None, op0=mybir.AluOpType.add)
                nc.sync.dma_start(out=y_out[:], in_=ysb[:])

            if dbg:
                nc.sync.dma_start(out=dbg_outs["d_ha"][:], in_=ha_full[:])
                nc.sync.dma_start(out=dbg_outs["d_hc"][:], in_=hc_full[:])
                nc.sync.dma_start(out=dbg_outs["d_ua"][:], in_=ua_full[:])
                nc.sync.dma_start(out=dbg_outs["d_uc"][:], in_=uc_loc[:])
                nc.sync.dma_start(out=dbg_outs["d_st"][:, 0:2],
                                  in_=stats_out_a[:])
                nc.sync.dma_start(out=dbg_outs["d_st"][:, 2:4],
                                  in_=stats_out_c[:])

    nc.compile()
    return nc


# ---------------------------------------------------------------------------
# entry point
# ---------------------------------------------------------------------------

def make_in_maps(cfg, inputs, pa, pcs, dec, scls):
    npdt = cfg.npdt
    f = lambda a: np.ascontiguousarray(np.asarray(a), dtype=np.float32)
    xc16 = f(inputs["x_customer"]).astype(npdt)
    xa16 = f(inputs["x_article"]).astype(npdt)
    wd1 = f(inputs["W_dec1"])
    w2 = f(inputs["W_dec2"]).reshape(-1)
    base = dict(
        xc=xc16, xa=xa16,
        W_msg1_ca=f(inputs["W_msg1_ca"]).astype(npdt),
        W_self1_a=f(inputs["W_self1_a"]).astype(npdt),
        W_msg1_ac=f(inputs["W_msg1_ac"]).astype(npdt),
        W_self1_c=f(inputs["W_self1_c"]).astype(npdt),
        W_msg2_ca=f(inputs["W_msg2_ca"]).astype(npdt),
        W_self2_a=f(inputs["W_self2_a"]).astype(npdt),
        W_msg2_ac=f(inputs["W_msg2_ac"]).astype(npdt),
        W_self2_c=f(inputs["W_self2_c"]).astype(npdt),
        Wd1c=wd1[:P].astype(npdt), Wd1a=wd1[P:].astype(npdt),
        w2rep=np.tile(w2.astype(npdt).reshape(1, P), (P, DGCH // P)),
        b1_a=f(inputs["b1_a"]).reshape(P, 1),
        b1_c=f(inputs["b1_c"]).reshape(P, 1),
        b2_a=f(inputs["b2_a"]).reshape(P, 1),
        b2_c=f(inputs["b2_c"]).reshape(P, 1),
        bn_gamma_c=f(inputs["bn_gamma_c"]).reshape(P, 1),
        bn_beta_c=f(inputs["bn_beta_c"]).reshape(P, 1),
        bn_gamma_a=f(inputs["bn_gamma_a"]).reshape(P, 1),
        bn_beta_a=f(inputs["bn_beta_a"]).reshape(P, 1),
        b_dec1=f(inputs["b_dec1"]).reshape(P, 1),
        b_dec2c=np.full((P, 1), float(np.asarray(inputs["b_dec2"]).item()),
                        np.float32),
    )
    in_maps = []
    for k in range(NCORES):
        m = dict(base)
        m["xaT"] = np.ascontiguousarray(
            xa16[k * cfg.apc:(k + 1) * cfg.apc].T)
        m["xcT"] = np.ascontiguousarray(
            xc16[k * cfg.cpc:(k + 1) * cfg.cpc].T)
        m["sclA"] = np.ascontiguousarray(np.tile(
            scls["scl_a"][k * cfg.apc:(k + 1) * cfg.apc][None, :],
            (P, 1)).astype(npdt))
        m["sclC"] = np.ascontiguousarray(np.tile(
            scls["scl_c"][k * cfg.cpc:(k + 1) * cfg.cpc][None, :],
            (P, 1)).astype(npdt))
        m["aggA_idx"] = pa.idx[k]
        m["aggA_dsc"] = pa.dsc[k]
        for h in range(2):
            m[f"aggC{h}_idx"] = pcs[h].idx[k]
            m[f"aggC{h}_dsc"] = pcs[h].dsc[k]
        m["dec_idx_c"] = dec["idx_c"][k]
        m["dec_idx_a"] = dec["idx_a"][k]
        in_maps.append(m)
    return in_maps


def run(cfg, inputs, trace=False, dbg=False):
    pa, pcs, dec, scls = prep_all(cfg, inputs)
    in_maps = make_in_maps(cfg, inputs, pa, pcs, dec, scls)
    nc = build_nc(cfg, pa, pcs, dec, dbg=dbg)
    res = run_bass_kernel_spmd(nc, in_maps, core_ids=list(range(NCORES)),
                               trace=trace)
    y = np.empty(cfg.e_lbl, np.float32)
    for k in range(NCORES):
        yl = res.results[k]["y"].T.reshape(-1)
        po = dec["out_pos"][k]
        vm = po >= 0
        y[po[vm]] = yl[vm]
    return y, res


def kernel(**inputs):
    cfg = Cfg()
    y, _ = run(cfg, inputs, trace=False)
    return y

